# revision 15
# baseline (speedup 1.0000x reference)
"""Distributed Trainium2 kernel for ChebConv (K=4) GNN message passing.

Math (matches the PyG ChebConv reference with sym norm, lambda_max=2):
    L_hat = -D^-1/2 A D^-1/2   (elementwise: ew[e] = -dinv[row]*dinv[col])
    Tx0 = x ; Tx1 = L_hat x ; Tx_{k+1} = 2 L_hat Tx_k - Tx_{k-1}
    y = tanh(sum_k Tx_k @ w_k + b) @ final_w + final_b

Key restructuring: the edge weight is separable, so
    L_hat z = -dinv ⊙ (A (dinv ⊙ z))
and the SpMM needs no per-edge weights: scale rows by dinv (dense), gather+sum
neighbor rows (ELL format), scale by -dinv.

Distribution: nodes are degree-sorted into 128-row tiles; tile stripes are
dealt round-robin to the 8 cores (load balance + uniform ELL widths across
cores, required for SPMD). Each Chebyshev step: every core computes
dinv ⊙ Tx_k for its rows, AllGathers the full scaled-feature table to HBM,
then gathers its padded neighbor lists from it with the DMAGatherAnt custom
instruction (int16 indices -> the table is addressed in a low half and a
high half; table row 0 and one high row are zero rows used for padding).
"""

import sys

sys.path.insert(0, "/opt/trn_rl_repo")

import numpy as np

N_NODES = 50000
N_EDGES = 800000
F = 64  # input features
H = 128  # hidden
K = 4  # chebyshev orders
NC = 8  # cores
P = 128  # partitions

WBMAX = 96  # max ELL chunk-slots per gather batch
LOWMAX = 32768  # int16 index reach (rows per table half)
DEBUG_DUMPS = False


def _refresh():
    """Recompute derived sizes from N_NODES (lets tests shrink the problem)."""
    global TPC, RPC, NTOT, NTAB, HIBASE, HIPAD
    TPC = -(-N_NODES // (P * NC))  # tiles per core
    RPC = TPC * P  # rows per core
    NTOT = NC * RPC  # padded node count
    NTAB = NTOT + 1 + P  # 1 zero row + nodes + zero tail
    HIBASE = LOWMAX  # first row of the high table half
    HIPAD = NTOT + 1 - HIBASE  # high-half index of a guaranteed zero row


_refresh()


def _wrap_idx(flat):
    """[K] flat int array -> [128, K/16] int16 SBUF image (16-wrapped, x8)."""
    s = flat.reshape(-1, 16).T.astype(np.int16)  # [16, K/16]
    return np.tile(s, (8, 1))


def _preprocess(edge_index):
    """Build permutation, split ELL structure and per-core arrays."""
    row = edge_index[0].astype(np.int64)
    col = edge_index[1].astype(np.int64)

    deg = np.bincount(row, minlength=N_NODES)
    dinv = np.zeros(N_NODES, np.float64)
    nz = deg > 0
    dinv[nz] = 1.0 / np.sqrt(deg[nz])
    dinv = dinv.astype(np.float32)

    # degree-sorted order; global tile t = sorted nodes [t*128, (t+1)*128)
    order = np.argsort(deg, kind="stable")
    t_of = np.arange(N_NODES) // P
    p_of = np.arange(N_NODES) % P
    new_of = (t_of % NC) * RPC + (t_of // NC) * P + p_of
    old2new = np.empty(N_NODES, np.int64)
    old2new[order] = new_of

    new_row = old2new[row]
    tabrow = old2new[col] + 1  # 1-based table rows (row 0 is the zero row)
    is_low = tabrow < LOWMAX

    def build_half(mask, values, pad):
        cnt = np.bincount(new_row[mask], minlength=NTOT)
        W = cnt.reshape(NC, TPC, P).max(axis=(0, 2)).astype(np.int64)
        W = np.maximum(W, 1)
        off = np.zeros(TPC + 1, np.int64)
        off[1:] = np.cumsum(W)
        starts = np.zeros(NTOT + 1, np.int64)
        starts[1:] = np.cumsum(cnt)
        sel = np.nonzero(mask)[0]
        o = np.argsort(new_row[sel], kind="stable")
        nr = new_row[sel][o]
        vv = values[sel][o]
        rank = np.arange(len(sel)) - starts[nr]
        core_e = nr // RPC
        rem = nr % RPC
        ti = rem // P
        p_e = rem % P
        colpad = np.full((NC, P, int(off[-1])), pad, np.int32)
        colpad[core_e, p_e, off[ti] + rank] = vv
        return W, off, colpad

    WL, offL, cp_lo = build_half(is_low, tabrow, 0)
    WH, offH, cp_hi = build_half(~is_low, tabrow - HIBASE, HIPAD)

    dinv_new = np.zeros(NTOT, np.float32)
    dinv_new[old2new] = dinv
    dinv_t = np.ascontiguousarray(dinv_new.reshape(NC, TPC, P).transpose(0, 2, 1))

    # batches of consecutive tiles with sum(WL+WH) <= WBMAX
    Wtot = WL + WH
    batches = []  # (t0, t1)
    i = 0
    while i < TPC:
        j = i + 1
        while j < TPC and Wtot[i : j + 1].sum() <= WBMAX:
            j += 1
        batches.append((i, j))
        i = j

    # int16 index images per core: per batch [ low image | high image ]
    tot_s = int(offL[-1] + offH[-1]) * 8
    idx_img = np.zeros((NC, P, tot_s), np.int16)
    binfo = []  # (t0, t1, CL, CH, soff)
    soff = 0
    for t0, t1 in batches:
        CL = int(offL[t1] - offL[t0])
        CH = int(offH[t1] - offH[t0])
        for c in range(NC):
            fl = cp_lo[c][:, offL[t0] : offL[t1]].T.reshape(-1)  # k = c*128+p
            fh = cp_hi[c][:, offH[t0] : offH[t1]].T.reshape(-1)
            if CL:
                idx_img[c][:, soff : soff + CL * 8] = _wrap_idx(fl)
            if CH:
                idx_img[c][:, soff + CL * 8 : soff + (CL + CH) * 8] = _wrap_idx(fh)
        binfo.append((t0, t1, CL, CH, soff))
        soff += (CL + CH) * 8
    assert soff == tot_s

    return dict(
        dinv=dinv,
        old2new=old2new,
        dinv_t=dinv_t,
        WL=WL,
        WH=WH,
        offL=offL,
        offH=offH,
        cp_lo=cp_lo,
        cp_hi=cp_hi,
        idx_img=idx_img,
        binfo=binfo,
        tot_s=tot_s,
    )


def _build_graph(pre):
    from concourse import bacc, mybir, tile
    import concourse.bass as bass
    from concourse.masks import make_identity

    f32 = mybir.dt.float32
    i16 = mybir.dt.int16
    WL, WH = pre["WL"], pre["WH"]
    offL, offH = pre["offL"], pre["offH"]
    binfo = pre["binfo"]
    tot_s = pre["tot_s"]

    nc = bacc.Bacc(None, target_bir_lowering=False, num_devices=NC)

    x_in = nc.declare_dram_parameter("x", [RPC, F], f32, isOutput=False)
    idx_in = nc.declare_dram_parameter("idximg", [P, tot_s], i16, isOutput=False)
    dinvt_in = nc.declare_dram_parameter("dinvt", [P, TPC], f32, isOutput=False)
    chebw_in = nc.declare_dram_parameter("cheb_w", [K, F, H], f32, isOutput=False)
    chebb_in = nc.declare_dram_parameter("cheb_b", [H, 1], f32, isOutput=False)
    finw_in = nc.declare_dram_parameter("final_w", [H, 1], f32, isOutput=False)
    finb_in = nc.declare_dram_parameter("final_b", [1, 1], f32, isOutput=False)
    y_out = nc.declare_dram_parameter("y", [1, RPC], f32, isOutput=True)

    tabA = nc.dram_tensor("tabA", [NTAB, F], f32, addr_space="Shared")
    tabB = nc.dram_tensor("tabB", [NTAB, F], f32, addr_space="Shared")
    tab_in = nc.dram_tensor("tabin", [RPC, F], f32)

    if DEBUG_DUMPS:
        dbg_tab1 = nc.declare_dram_parameter("dbg_tab1", [NTAB, F], f32, isOutput=True)
        dbg_g = nc.declare_dram_parameter("dbg_g", [P, WBMAX * F], f32, isOutput=True)
        dbg_tx1 = nc.declare_dram_parameter("dbg_tx1", [P, TPC * F], f32, isOutput=True)

    rg = [list(range(NC))]

    with tile.TileContext(nc) as tc:
        with (
            tc.tile_pool(name="persist", bufs=1) as persist,
            tc.tile_pool(name="work", bufs=3) as work,
            tc.tile_pool(name="gpool", bufs=2) as gpool,
            tc.tile_pool(name="psum", bufs=3, space="PSUM") as psum,
            tc.tile_pool(name="psum_h", bufs=2, space="PSUM") as psum_h,
            tc.tile_pool(name="psum_y", bufs=2, space="PSUM") as psum_y,
        ):
            # ---- persistent tiles ----
            ident = persist.tile([P, P], f32)
            make_identity(nc, ident[:])

            dinv_sb = persist.tile([P, TPC], f32)
            nc.sync.dma_start(out=dinv_sb[:], in_=dinvt_in[:, :])
            ndinv_sb = persist.tile([P, TPC], f32)
            nc.vector.tensor_scalar_mul(ndinv_sb[:], dinv_sb[:], -1.0)
            n2dinv_sb = persist.tile([P, TPC], f32)
            nc.vector.tensor_scalar_mul(n2dinv_sb[:], dinv_sb[:], -2.0)

            wc01 = persist.tile([2 * F, H], f32)
            nc.sync.dma_start(out=wc01[0:F, :], in_=chebw_in[0])
            nc.sync.dma_start(out=wc01[F : 2 * F, :], in_=chebw_in[1])
            wc23 = persist.tile([2 * F, H], f32)
            nc.sync.dma_start(out=wc23[0:F, :], in_=chebw_in[2])
            nc.sync.dma_start(out=wc23[F : 2 * F, :], in_=chebw_in[3])

            chebb_sb = persist.tile([H, 1], f32)
            nc.sync.dma_start(out=chebb_sb[:], in_=chebb_in[:, :])
            finw_sb = persist.tile([H, 1], f32)
            nc.sync.dma_start(out=finw_sb[:], in_=finw_in[:, :])
            finb_sb = persist.tile([1, 1], f32)
            nc.sync.dma_start(out=finb_sb[:], in_=finb_in[:, :])

            # feature-major stashes of Tx_k^T for the final projection
            txt01 = persist.tile([P, TPC * P], f32)  # parts 0:64 Tx0^T, 64:128 Tx1^T
            txt23 = persist.tile([P, TPC * P], f32)
            # node-major Tx1 (needed by the k=3 recursion)
            tx1slab = persist.tile([P, TPC * F], f32)
            # final output row
            y_sb = persist.tile([1, RPC], f32)

            # ---- zero rows of both tables (row 0 and the tail) ----
            zt = work.tile([P, F], f32, tag="xt")
            nc.vector.memset(zt[:], 0.0)
            for tab in (tabA, tabB):
                nc.sync.dma_start(out=tab[0:1, :], in_=zt[0:1, :])
                nc.sync.dma_start(out=tab[NTOT + 1 : NTAB, :], in_=zt[0:P, :])

            # ---- step 0: table1 = dinv*x, stash Tx0^T ----
            for i in range(TPC):
                rows = slice(i * P, (i + 1) * P)
                xt = work.tile([P, F], f32, tag="xt")
                nc.sync.dma_start(out=xt[:], in_=x_in[rows, :])
                tabt = work.tile([P, F], f32, tag="tabt")
                nc.vector.tensor_scalar_mul(tabt[:], xt[:], dinv_sb[:, i : i + 1])
                nc.sync.dma_start(out=tab_in[rows, :], in_=tabt[:])
                ps = psum.tile([F, P], f32)
                nc.tensor.transpose(out=ps[:], in_=xt[:], identity=ident[:])
                nc.vector.tensor_copy(out=txt01[0:F, rows], in_=ps[:])

            def allgather(dst):
                nc.gpsimd.collective_compute(
                    "AllGather",
                    mybir.AluOpType.bypass,
                    replica_groups=rg,
                    ins=[tab_in[:, :].opt()],
                    outs=[dst[1 : NTOT + 1, :].opt()],
                )

            allgather(tabA)

            if DEBUG_DUMPS:
                nc.sync.dma_start(out=dbg_tab1[:, :], in_=tabA[:, :])

            # ---- chebyshev steps ----
            for s in (1, 2, 3):
                tab = tabA if s != 2 else tabB
                for t0, t1, CL, CH, soff in binfo:
                    sb_cols = (CL + CH) * 8
                    idxt = work.tile([P, WBMAX * 8], i16, tag="idx")
                    nc.sync.dma_start(
                        out=idxt[:, :sb_cols], in_=idx_in[:, soff : soff + sb_cols]
                    )
                    G = gpool.tile([P, WBMAX * F], f32, tag="G")
                    if CL:
                        nc.gpsimd.dma_gather(
                            out_ap=G[:, : CL * F].rearrange("p (c f) -> p c f", f=F),
                            in_ap=tab[:, :],
                            idxs_ap=idxt[:, : CL * 8],
                            num_idxs=CL * P,
                            num_idxs_reg=CL * P,
                            elem_size=F,
                            single_packet=False,
                        )
                    if CH:
                        nc.gpsimd.dma_gather(
                            out_ap=G[:, CL * F : (CL + CH) * F].rearrange(
                                "p (c f) -> p c f", f=F
                            ),
                            in_ap=tab[HIBASE:NTAB, :],
                            idxs_ap=idxt[:, CL * 8 : (CL + CH) * 8],
                            num_idxs=CH * P,
                            num_idxs_reg=CH * P,
                            elem_size=F,
                            single_packet=False,
                        )
                    if DEBUG_DUMPS and s == 1 and t0 == 0:
                        nc.sync.dma_start(
                            out=dbg_g[:, : (CL + CH) * F], in_=G[:, : (CL + CH) * F]
                        )
                    for i in range(t0, t1):
                        wl, wh = int(WL[i]), int(WH[i])
                        lo = int(offL[i] - offL[t0])
                        hi = CL + int(offH[i] - offH[t0])
                        rows = slice(i * P, (i + 1) * P)
                        fcols = slice(i * F, (i + 1) * F)
                        S = work.tile([P, F, 1], f32, tag="S")
                        nc.vector.reduce_sum(
                            out=S[:],
                            in_=G[:, lo * F : (lo + wl) * F].rearrange(
                                "p (w f) -> p f w", f=F
                            ),
                            axis=mybir.AxisListType.X,
                        )
                        S2 = work.tile([P, F, 1], f32, tag="S2")
                        nc.vector.reduce_sum(
                            out=S2[:],
                            in_=G[:, hi * F : (hi + wh) * F].rearrange(
                                "p (w f) -> p f w", f=F
                            ),
                            axis=mybir.AxisListType.X,
                        )
                        Ssum = work.tile([P, F], f32, tag="Ssum")
                        nc.vector.tensor_add(Ssum[:], S[:, :, 0], S2[:, :, 0])
                        if s == 1:
                            nc.vector.tensor_scalar_mul(
                                tx1slab[:, fcols], Ssum[:], ndinv_sb[:, i : i + 1]
                            )
                            txk = tx1slab[:, fcols]
                        else:
                            tmp = work.tile([P, F], f32, tag="tmp")
                            nc.vector.tensor_scalar_mul(
                                tmp[:], Ssum[:], n2dinv_sb[:, i : i + 1]
                            )
                            txk_t = work.tile([P, F], f32, tag="txk")
                            if s == 2:
                                xt = work.tile([P, F], f32, tag="xt")
                                nc.sync.dma_start(out=xt[:], in_=x_in[rows, :])
                                nc.vector.tensor_sub(txk_t[:], tmp[:], xt[:])
                            else:
                                nc.vector.tensor_sub(
                                    txk_t[:], tmp[:], tx1slab[:, fcols]
                                )
                            txk = txk_t[:]
                        if s < 3:
                            tabt = work.tile([P, F], f32, tag="tabt")
                            nc.vector.tensor_scalar_mul(
                                tabt[:], txk, dinv_sb[:, i : i + 1]
                            )
                            nc.sync.dma_start(out=tab_in[rows, :], in_=tabt[:])
                        ps = psum.tile([F, P], f32)
                        nc.tensor.transpose(out=ps[:], in_=txk, identity=ident[:])
                        dst = txt01 if s == 1 else txt23
                        pr = slice(F, 2 * F) if s in (1, 3) else slice(0, F)
                        nc.vector.tensor_copy(out=dst[pr, rows], in_=ps[:])

                if s == 1:
                    allgather(tabB)
                elif s == 2:
                    allgather(tabA)

            if DEBUG_DUMPS:
                nc.sync.dma_start(out=dbg_tx1[:, :], in_=tx1slab[:])

            # ---- projection + tanh + final linear ----
            for i in range(TPC):
                rows = slice(i * P, (i + 1) * P)
                hps = psum_h.tile([H, P], f32)
                nc.tensor.matmul(
                    out=hps[:], lhsT=wc01[:], rhs=txt01[:, rows], start=True, stop=False
                )
                nc.tensor.matmul(
                    out=hps[:], lhsT=wc23[:], rhs=txt23[:, rows], start=False, stop=True
                )
                hT = work.tile([H, P], f32, tag="hT")
                nc.scalar.activation(
                    out=hT[:],
                    in_=hps[:],
                    func=mybir.ActivationFunctionType.Tanh,
                    bias=chebb_sb[:, 0:1],
                    scale=1.0,
                )
                yps = psum_y.tile([1, P], f32, tag="yps")
                nc.tensor.matmul(
                    out=yps[:], lhsT=finw_sb[:], rhs=hT[:], start=True, stop=True
                )
                nc.vector.tensor_scalar_add(y_sb[0:1, rows], yps[:], finb_sb[0:1, 0:1])

            nc.sync.dma_start(out=y_out[:, :], in_=y_sb[:])

    nc.finalize()
    return nc


def run(features, edge_index, cheb_w, cheb_b, final_w, final_b, **spmd_kwargs):
    """Build + compile + run; returns (y, BassKernelResults)."""
    from concourse.bass_utils import run_bass_kernel_spmd

    features = np.asarray(features, np.float32)
    edge_index = np.asarray(edge_index)
    cheb_w = np.asarray(cheb_w, np.float32)
    cheb_b = np.asarray(cheb_b, np.float32)
    final_w = np.asarray(final_w, np.float32)
    final_b = np.asarray(final_b, np.float32)

    pre = _preprocess(edge_index)
    nc = _build_graph(pre)

    old2new = pre["old2new"]
    x_new = np.zeros((NTOT, F), np.float32)
    x_new[old2new] = features
    x_new = x_new.reshape(NC, RPC, F)

    in_maps = []
    for c in range(NC):
        in_maps.append(
            dict(
                x=np.ascontiguousarray(x_new[c]),
                idximg=np.ascontiguousarray(pre["idx_img"][c]),
                dinvt=np.ascontiguousarray(pre["dinv_t"][c]),
                cheb_w=cheb_w,
                cheb_b=cheb_b.reshape(H, 1),
                final_w=final_w.reshape(H, 1),
                final_b=final_b.reshape(1, 1),
            )
        )

    res = run_bass_kernel_spmd(nc, in_maps, core_ids=list(range(NC)), **spmd_kwargs)
    y_new = np.concatenate([r["y"].reshape(-1) for r in res.results])
    return y_new[old2new].astype(np.float32), res


def kernel(features, edge_index, cheb_w, cheb_b, final_w, final_b):
    y, _ = run(features, edge_index, cheb_w, cheb_b, final_w, final_b)
    return y


# revision 20
# speedup vs baseline: 1.0608x; 1.0608x over previous
"""Distributed Trainium2 kernel for ChebConv (K=4) GNN message passing.

Math (matches the PyG ChebConv reference with sym norm, lambda_max=2):
    L_hat = -D^-1/2 A D^-1/2   (elementwise: ew[e] = -dinv[row]*dinv[col])
    Tx0 = x ; Tx1 = L_hat x ; Tx_{k+1} = 2 L_hat Tx_k - Tx_{k-1}
    y = tanh(sum_k Tx_k @ w_k + b) @ final_w + final_b

Key restructuring: the edge weight is separable, so
    L_hat z = -dinv ⊙ (A (dinv ⊙ z))
and the SpMM needs no per-edge weights: scale rows by dinv (dense), gather+sum
neighbor rows (ELL format), scale by -dinv.

Distribution: nodes are degree-sorted into 128-row tiles; tile stripes are
dealt round-robin to the 8 cores (load balance + uniform ELL widths across
cores, required for SPMD). Each Chebyshev step: every core computes
dinv ⊙ Tx_k for its rows, AllGathers the full scaled-feature table to HBM,
then gathers its padded neighbor lists from it with the DMAGatherAnt custom
instruction (int16 indices -> the table is addressed in a low half and a
high half; table row 0 and one high row are zero rows used for padding).
"""

import sys

sys.path.insert(0, "/opt/trn_rl_repo")

import numpy as np

N_NODES = 50000
N_EDGES = 800000
F = 64  # input features
H = 128  # hidden
K = 4  # chebyshev orders
NC = 8  # cores
P = 128  # partitions

WBMAX = 96  # max ELL chunk-slots per gather batch
LOWMAX = 32768  # int16 index reach (rows per table half)
DEBUG_DUMPS = False


def _refresh():
    """Recompute derived sizes from N_NODES (lets tests shrink the problem)."""
    global TPC, RPC, NTOT, NTAB, HIBASE, HIPAD
    TPC = -(-N_NODES // (P * NC))  # tiles per core
    RPC = TPC * P  # rows per core
    NTOT = NC * RPC  # padded node count
    NTAB = NTOT + 1 + P  # 1 zero row + nodes + zero tail
    HIBASE = NTAB - LOWMAX  # first row of the high table half (overlap window)
    HIPAD = NTOT + 1 - HIBASE  # high-half index of a guaranteed zero row


_refresh()


def _wrap_idx(flat):
    """[K] flat int array -> [128, K/16] int16 SBUF image (16-wrapped, x8)."""
    s = flat.reshape(-1, 16).T.astype(np.int16)  # [16, K/16]
    return np.tile(s, (8, 1))


def _preprocess(edge_index):
    """Build permutation, split ELL structure and per-core arrays."""
    row = edge_index[0].astype(np.int64)
    col = edge_index[1].astype(np.int64)

    deg = np.bincount(row, minlength=N_NODES)
    dinv = np.zeros(N_NODES, np.float64)
    nz = deg > 0
    dinv[nz] = 1.0 / np.sqrt(deg[nz])
    dinv = dinv.astype(np.float32)

    # degree-sorted order; global tile t = sorted nodes [t*128, (t+1)*128)
    order = np.argsort(deg, kind="stable")
    t_of = np.arange(N_NODES) // P
    p_of = np.arange(N_NODES) % P
    new_of = (t_of % NC) * RPC + (t_of // NC) * P + p_of
    old2new = np.empty(N_NODES, np.int64)
    old2new[order] = new_of

    new_row = old2new[row]
    tabrow = old2new[col] + 1  # 1-based table rows (row 0 is the zero row)

    # Rows in [HIBASE, LOWMAX) are reachable from either table half; assign
    # them per destination row to balance the two groups (minimizes the
    # padded ELL widths, which the Q7 descriptor-gen time scales with).
    lo_forced = tabrow < HIBASE
    hi_forced = tabrow >= LOWMAX
    flex = ~(lo_forced | hi_forced)

    nlo = np.bincount(new_row[lo_forced], minlength=NTOT)
    nhi = np.bincount(new_row[hi_forced], minlength=NTOT)
    nflex = np.bincount(new_row[flex], minlength=NTOT)
    x = np.clip((nhi + nflex - nlo + 1) // 2, 0, nflex)  # flex sent to LOW

    # rank of each flex edge within its row's flex group
    fsel = np.nonzero(flex)[0]
    fo = np.argsort(new_row[fsel], kind="stable")
    fstarts = np.zeros(NTOT + 1, np.int64)
    fstarts[1:] = np.cumsum(nflex)
    frank = np.arange(len(fsel)) - fstarts[new_row[fsel][fo]]
    flex_low = np.zeros(N_EDGES, bool)
    flex_low[fsel[fo]] = frank < x[new_row[fsel][fo]]

    is_low = lo_forced | flex_low

    def build_half(mask, values, pad):
        cnt = np.bincount(new_row[mask], minlength=NTOT)
        W = cnt.reshape(NC, TPC, P).max(axis=(0, 2)).astype(np.int64)
        W = np.maximum(W, 1)
        off = np.zeros(TPC + 1, np.int64)
        off[1:] = np.cumsum(W)
        starts = np.zeros(NTOT + 1, np.int64)
        starts[1:] = np.cumsum(cnt)
        sel = np.nonzero(mask)[0]
        o = np.argsort(new_row[sel], kind="stable")
        nr = new_row[sel][o]
        vv = values[sel][o]
        rank = np.arange(len(sel)) - starts[nr]
        core_e = nr // RPC
        rem = nr % RPC
        ti = rem // P
        p_e = rem % P
        colpad = np.full((NC, P, int(off[-1])), pad, np.int32)
        colpad[core_e, p_e, off[ti] + rank] = vv
        return W, off, colpad

    WL, offL, cp_lo = build_half(is_low, tabrow, 0)
    WH, offH, cp_hi = build_half(~is_low, tabrow - HIBASE, HIPAD)

    dinv_new = np.zeros(NTOT, np.float32)
    dinv_new[old2new] = dinv
    dinv_t = np.ascontiguousarray(dinv_new.reshape(NC, TPC, P).transpose(0, 2, 1))

    # batches of consecutive tiles with sum(WL+WH) <= WBMAX
    Wtot = WL + WH
    batches = []  # (t0, t1)
    i = 0
    while i < TPC:
        j = i + 1
        while j < TPC and Wtot[i : j + 1].sum() <= WBMAX:
            j += 1
        batches.append((i, j))
        i = j

    # int16 index images per core: per batch [ low image | high image ]
    tot_s = int(offL[-1] + offH[-1]) * 8
    idx_img = np.zeros((NC, P, tot_s), np.int16)
    binfo = []  # (t0, t1, CL, CH, soff)
    soff = 0
    for t0, t1 in batches:
        CL = int(offL[t1] - offL[t0])
        CH = int(offH[t1] - offH[t0])
        for c in range(NC):
            fl = cp_lo[c][:, offL[t0] : offL[t1]].T.reshape(-1)  # k = c*128+p
            fh = cp_hi[c][:, offH[t0] : offH[t1]].T.reshape(-1)
            if CL:
                idx_img[c][:, soff : soff + CL * 8] = _wrap_idx(fl)
            if CH:
                idx_img[c][:, soff + CL * 8 : soff + (CL + CH) * 8] = _wrap_idx(fh)
        binfo.append((t0, t1, CL, CH, soff))
        soff += (CL + CH) * 8
    assert soff == tot_s

    return dict(
        dinv=dinv,
        old2new=old2new,
        dinv_t=dinv_t,
        WL=WL,
        WH=WH,
        offL=offL,
        offH=offH,
        cp_lo=cp_lo,
        cp_hi=cp_hi,
        idx_img=idx_img,
        binfo=binfo,
        tot_s=tot_s,
    )


def _build_graph(pre):
    from concourse import bacc, mybir, tile
    import concourse.bass as bass
    from concourse.masks import make_identity

    f32 = mybir.dt.float32
    i16 = mybir.dt.int16
    WL, WH = pre["WL"], pre["WH"]
    offL, offH = pre["offL"], pre["offH"]
    binfo = pre["binfo"]
    tot_s = pre["tot_s"]

    nc = bacc.Bacc(None, target_bir_lowering=False, num_devices=NC)

    x_in = nc.declare_dram_parameter("x", [RPC, F], f32, isOutput=False)
    idx_in = nc.declare_dram_parameter("idximg", [P, tot_s], i16, isOutput=False)
    dinvt_in = nc.declare_dram_parameter("dinvt", [P, TPC], f32, isOutput=False)
    chebw_in = nc.declare_dram_parameter("cheb_w", [K, F, H], f32, isOutput=False)
    chebb_in = nc.declare_dram_parameter("cheb_b", [H, 1], f32, isOutput=False)
    finw_in = nc.declare_dram_parameter("final_w", [H, 1], f32, isOutput=False)
    finb_in = nc.declare_dram_parameter("final_b", [1, 1], f32, isOutput=False)
    y_out = nc.declare_dram_parameter("y", [1, RPC], f32, isOutput=True)

    tabA = nc.dram_tensor("tabA", [NTAB, F], f32, addr_space="Shared")
    tabB = nc.dram_tensor("tabB", [NTAB, F], f32, addr_space="Shared")
    tab_in = nc.dram_tensor("tabin", [RPC, F], f32)

    if DEBUG_DUMPS:
        dbg_tab1 = nc.declare_dram_parameter("dbg_tab1", [NTAB, F], f32, isOutput=True)
        dbg_g = nc.declare_dram_parameter("dbg_g", [P, WBMAX * F], f32, isOutput=True)
        dbg_tx1 = nc.declare_dram_parameter("dbg_tx1", [P, TPC * F], f32, isOutput=True)

    rg = [list(range(NC))]

    with tile.TileContext(nc) as tc:
        with (
            tc.tile_pool(name="persist", bufs=1) as persist,
            tc.tile_pool(name="work", bufs=4) as work,
            tc.tile_pool(name="gpool", bufs=3) as gpool,
            tc.tile_pool(name="psum", bufs=3, space="PSUM") as psum,
            tc.tile_pool(name="psum_h", bufs=2, space="PSUM") as psum_h,
            tc.tile_pool(name="psum_y", bufs=2, space="PSUM") as psum_y,
        ):
            # ---- persistent tiles ----
            ident = persist.tile([P, P], f32)
            make_identity(nc, ident[:])

            dinv_sb = persist.tile([P, TPC], f32)
            nc.sync.dma_start(out=dinv_sb[:], in_=dinvt_in[:, :])
            ndinv_sb = persist.tile([P, TPC], f32)
            nc.vector.tensor_scalar_mul(ndinv_sb[:], dinv_sb[:], -1.0)
            n2dinv_sb = persist.tile([P, TPC], f32)
            nc.vector.tensor_scalar_mul(n2dinv_sb[:], dinv_sb[:], -2.0)

            wc01 = persist.tile([2 * F, H], f32)
            nc.sync.dma_start(out=wc01[0:F, :], in_=chebw_in[0])
            nc.sync.dma_start(out=wc01[F : 2 * F, :], in_=chebw_in[1])
            wc23 = persist.tile([2 * F, H], f32)
            nc.sync.dma_start(out=wc23[0:F, :], in_=chebw_in[2])
            nc.sync.dma_start(out=wc23[F : 2 * F, :], in_=chebw_in[3])

            chebb_sb = persist.tile([H, 1], f32)
            nc.sync.dma_start(out=chebb_sb[:], in_=chebb_in[:, :])
            finw_sb = persist.tile([H, 1], f32)
            nc.sync.dma_start(out=finw_sb[:], in_=finw_in[:, :])
            finb_sb = persist.tile([1, 1], f32)
            nc.sync.dma_start(out=finb_sb[:], in_=finb_in[:, :])

            # feature-major stashes of Tx_k^T for the final projection
            txt01 = persist.tile([P, TPC * P], f32)  # parts 0:64 Tx0^T, 64:128 Tx1^T
            txt23 = persist.tile([P, TPC * P], f32)
            # node-major Tx1 (needed by the k=3 recursion)
            tx1slab = persist.tile([P, TPC * F], f32)
            # final output row
            y_sb = persist.tile([1, RPC], f32)

            # ---- zero rows of both tables (row 0 and the tail) ----
            zt = work.tile([P, F], f32, tag="xt")
            nc.vector.memset(zt[:], 0.0)
            for tab in (tabA, tabB):
                nc.sync.dma_start(out=tab[0:1, :], in_=zt[0:1, :])
                nc.sync.dma_start(out=tab[NTOT + 1 : NTAB, :], in_=zt[0:P, :])

            # ---- step 0: table1 = dinv*x, stash Tx0^T ----
            for i in range(TPC):
                rows = slice(i * P, (i + 1) * P)
                xt = work.tile([P, F], f32, tag="xt")
                nc.sync.dma_start(out=xt[:], in_=x_in[rows, :])
                tabt = work.tile([P, F], f32, tag="tabt")
                nc.scalar.activation(
                    out=tabt[:],
                    in_=xt[:],
                    func=mybir.ActivationFunctionType.Copy,
                    scale=dinv_sb[:, i : i + 1],
                )
                nc.sync.dma_start(out=tab_in[rows, :], in_=tabt[:])
                ps = psum.tile([F, P], f32)
                nc.tensor.transpose(out=ps[:], in_=xt[:], identity=ident[:])
                nc.scalar.activation(
                    out=txt01[0:F, rows],
                    in_=ps[:],
                    func=mybir.ActivationFunctionType.Copy,
                )

            def allgather(dst):
                nc.gpsimd.collective_compute(
                    "AllGather",
                    mybir.AluOpType.bypass,
                    replica_groups=rg,
                    ins=[tab_in[:, :].opt()],
                    outs=[dst[1 : NTOT + 1, :].opt()],
                )

            allgather(tabA)

            if DEBUG_DUMPS:
                nc.sync.dma_start(out=dbg_tab1[:, :], in_=tabA[:, :])

            # ---- chebyshev steps ----
            for s in (1, 2, 3):
                tab = tabA if s != 2 else tabB
                for t0, t1, CL, CH, soff in binfo:
                    sb_cols = (CL + CH) * 8
                    idxt = work.tile([P, WBMAX * 8], i16, tag="idx")
                    nc.sync.dma_start(
                        out=idxt[:, :sb_cols], in_=idx_in[:, soff : soff + sb_cols]
                    )
                    G = gpool.tile([P, WBMAX * F], f32, tag="G")
                    if CL:
                        nc.gpsimd.dma_gather(
                            out_ap=G[:, : CL * F].rearrange("p (c f) -> p c f", f=F),
                            in_ap=tab[:, :],
                            idxs_ap=idxt[:, : CL * 8],
                            num_idxs=CL * P,
                            num_idxs_reg=CL * P,
                            elem_size=F,
                            single_packet=False,
                        )
                    if CH:
                        nc.gpsimd.dma_gather(
                            out_ap=G[:, CL * F : (CL + CH) * F].rearrange(
                                "p (c f) -> p c f", f=F
                            ),
                            in_ap=tab[HIBASE:NTAB, :],
                            idxs_ap=idxt[:, CL * 8 : (CL + CH) * 8],
                            num_idxs=CH * P,
                            num_idxs_reg=CH * P,
                            elem_size=F,
                            single_packet=False,
                        )
                    if DEBUG_DUMPS and s == 1 and t0 == 0:
                        nc.sync.dma_start(
                            out=dbg_g[:, : (CL + CH) * F], in_=G[:, : (CL + CH) * F]
                        )
                    for i in range(t0, t1):
                        wl, wh = int(WL[i]), int(WH[i])
                        lo = int(offL[i] - offL[t0])
                        hi = CL + int(offH[i] - offH[t0])
                        rows = slice(i * P, (i + 1) * P)
                        fcols = slice(i * F, (i + 1) * F)
                        S = work.tile([P, F, 1], f32, tag="S")
                        nc.vector.reduce_sum(
                            out=S[:],
                            in_=G[:, lo * F : (lo + wl) * F].rearrange(
                                "p (w f) -> p f w", f=F
                            ),
                            axis=mybir.AxisListType.X,
                        )
                        S2 = work.tile([P, F, 1], f32, tag="S2")
                        nc.vector.reduce_sum(
                            out=S2[:],
                            in_=G[:, hi * F : (hi + wh) * F].rearrange(
                                "p (w f) -> p f w", f=F
                            ),
                            axis=mybir.AxisListType.X,
                        )
                        Ssum = work.tile([P, F], f32, tag="Ssum")
                        nc.vector.tensor_add(Ssum[:], S[:, :, 0], S2[:, :, 0])
                        if s == 1:
                            nc.vector.tensor_scalar_mul(
                                tx1slab[:, fcols], Ssum[:], ndinv_sb[:, i : i + 1]
                            )
                            txk = tx1slab[:, fcols]
                        else:
                            tmp = work.tile([P, F], f32, tag="tmp")
                            nc.vector.tensor_scalar_mul(
                                tmp[:], Ssum[:], n2dinv_sb[:, i : i + 1]
                            )
                            txk_t = work.tile([P, F], f32, tag="txk")
                            if s == 2:
                                xt = work.tile([P, F], f32, tag="xt")
                                nc.sync.dma_start(out=xt[:], in_=x_in[rows, :])
                                nc.vector.tensor_sub(txk_t[:], tmp[:], xt[:])
                            else:
                                nc.vector.tensor_sub(
                                    txk_t[:], tmp[:], tx1slab[:, fcols]
                                )
                            txk = txk_t[:]
                        if s < 3:
                            tabt = work.tile([P, F], f32, tag="tabt")
                            nc.scalar.activation(
                                out=tabt[:],
                                in_=txk,
                                func=mybir.ActivationFunctionType.Copy,
                                scale=dinv_sb[:, i : i + 1],
                            )
                            nc.sync.dma_start(out=tab_in[rows, :], in_=tabt[:])
                        ps = psum.tile([F, P], f32)
                        nc.tensor.transpose(out=ps[:], in_=txk, identity=ident[:])
                        dst = txt01 if s == 1 else txt23
                        pr = slice(F, 2 * F) if s in (1, 3) else slice(0, F)
                        nc.scalar.activation(
                            out=dst[pr, rows],
                            in_=ps[:],
                            func=mybir.ActivationFunctionType.Copy,
                        )

                if s == 1:
                    allgather(tabB)
                elif s == 2:
                    allgather(tabA)

            if DEBUG_DUMPS:
                nc.sync.dma_start(out=dbg_tx1[:, :], in_=tx1slab[:])

            # ---- projection + tanh + final linear ----
            for i in range(TPC):
                rows = slice(i * P, (i + 1) * P)
                hps = psum_h.tile([H, P], f32)
                nc.tensor.matmul(
                    out=hps[:], lhsT=wc01[:], rhs=txt01[:, rows], start=True, stop=False
                )
                nc.tensor.matmul(
                    out=hps[:], lhsT=wc23[:], rhs=txt23[:, rows], start=False, stop=True
                )
                hT = work.tile([H, P], f32, tag="hT")
                nc.scalar.activation(
                    out=hT[:],
                    in_=hps[:],
                    func=mybir.ActivationFunctionType.Tanh,
                    bias=chebb_sb[:, 0:1],
                    scale=1.0,
                )
                yps = psum_y.tile([1, P], f32, tag="yps")
                nc.tensor.matmul(
                    out=yps[:], lhsT=finw_sb[:], rhs=hT[:], start=True, stop=True
                )
                nc.vector.tensor_scalar_add(y_sb[0:1, rows], yps[:], finb_sb[0:1, 0:1])

            nc.sync.dma_start(out=y_out[:, :], in_=y_sb[:])

    nc.finalize()
    return nc


def run(features, edge_index, cheb_w, cheb_b, final_w, final_b, **spmd_kwargs):
    """Build + compile + run; returns (y, BassKernelResults)."""
    from concourse.bass_utils import run_bass_kernel_spmd

    features = np.asarray(features, np.float32)
    edge_index = np.asarray(edge_index)
    cheb_w = np.asarray(cheb_w, np.float32)
    cheb_b = np.asarray(cheb_b, np.float32)
    final_w = np.asarray(final_w, np.float32)
    final_b = np.asarray(final_b, np.float32)

    pre = _preprocess(edge_index)
    nc = _build_graph(pre)

    old2new = pre["old2new"]
    x_new = np.zeros((NTOT, F), np.float32)
    x_new[old2new] = features
    x_new = x_new.reshape(NC, RPC, F)

    in_maps = []
    for c in range(NC):
        in_maps.append(
            dict(
                x=np.ascontiguousarray(x_new[c]),
                idximg=np.ascontiguousarray(pre["idx_img"][c]),
                dinvt=np.ascontiguousarray(pre["dinv_t"][c]),
                cheb_w=cheb_w,
                cheb_b=cheb_b.reshape(H, 1),
                final_w=final_w.reshape(H, 1),
                final_b=final_b.reshape(1, 1),
            )
        )

    res = run_bass_kernel_spmd(nc, in_maps, core_ids=list(range(NC)), **spmd_kwargs)
    y_new = np.concatenate([r["y"].reshape(-1) for r in res.results])
    return y_new[old2new].astype(np.float32), res


def kernel(features, edge_index, cheb_w, cheb_b, final_w, final_b):
    y, _ = run(features, edge_index, cheb_w, cheb_b, final_w, final_b)
    return y


# revision 28
# speedup vs baseline: 1.3361x; 1.2596x over previous
"""Distributed Trainium2 kernel for ChebConv (K=4) GNN message passing.

Math (matches the PyG ChebConv reference with sym norm, lambda_max=2):
    L_hat = -D^-1/2 A D^-1/2   (elementwise: ew[e] = -dinv[row]*dinv[col])
    Tx0 = x ; Tx1 = L_hat x ; Tx_{k+1} = 2 L_hat Tx_k - Tx_{k-1}
    y = tanh(sum_k Tx_k @ w_k + b) @ final_w + final_b

Key restructuring: the edge weight is separable, so
    L_hat z = -dinv ⊙ (A (dinv ⊙ z))
and the SpMM needs no per-edge weights: scale rows by dinv (dense), gather+sum
neighbor rows (ELL format), scale by -dinv.

Distribution: nodes are degree-sorted into 128-row tiles; tile stripes are
dealt round-robin to the 8 cores (load balance + uniform ELL widths across
cores, required for SPMD). Each Chebyshev step: every core computes
dinv ⊙ Tx_k for its rows, AllGathers the full scaled-feature table to HBM,
then gathers its padded neighbor lists from it with the DMAGatherAnt custom
instruction (int16 indices -> the table is addressed in a low half and a
high half; table row 0 and one high row are zero rows used for padding).
"""

import sys

sys.path.insert(0, "/opt/trn_rl_repo")

import numpy as np

N_NODES = 50000
N_EDGES = 800000
F = 64  # input features
H = 128  # hidden
K = 4  # chebyshev orders
NC = 8  # cores
P = 128  # partitions

WBMAX = 96  # max ELL chunk-slots per gather batch
LOWMAX = 32768  # int16 index reach (rows per table half)
DEBUG_DUMPS = False


def _refresh():
    """Recompute derived sizes from N_NODES (lets tests shrink the problem)."""
    global TPC, RPC, NTOT, NTAB, HIBASE, HIPAD
    TPC = -(-N_NODES // (P * NC))  # tiles per core
    RPC = TPC * P  # rows per core
    NTOT = NC * RPC  # padded node count
    NTAB = NTOT + 1 + P  # 1 zero row + nodes + zero tail
    HIBASE = NTAB - LOWMAX  # first row of the high table half (overlap window)
    HIPAD = NTOT + 1 - HIBASE  # high-half index of a guaranteed zero row


_refresh()


def _wrap_idx(flat):
    """[K] flat int array -> [128, K/16] int16 SBUF image (16-wrapped, x8)."""
    s = flat.reshape(-1, 16).T.astype(np.int16)  # [16, K/16]
    return np.tile(s, (8, 1))


def _preprocess(edge_index):
    """Build permutation, split ELL structure and per-core arrays."""
    row = edge_index[0].astype(np.int64)
    col = edge_index[1].astype(np.int64)

    deg = np.bincount(row, minlength=N_NODES)
    dinv = np.zeros(N_NODES, np.float64)
    nz = deg > 0
    dinv[nz] = 1.0 / np.sqrt(deg[nz])
    dinv = dinv.astype(np.float32)

    # degree-sorted order; global tile t = sorted nodes [t*128, (t+1)*128)
    order = np.argsort(deg, kind="stable")
    t_of = np.arange(N_NODES) // P
    p_of = np.arange(N_NODES) % P
    new_of = (t_of % NC) * RPC + (t_of // NC) * P + p_of
    old2new = np.empty(N_NODES, np.int64)
    old2new[order] = new_of

    new_row = old2new[row]
    tabrow = old2new[col] + 1  # 1-based table rows (row 0 is the zero row)

    # Rows in [HIBASE, LOWMAX) are reachable from either table half; assign
    # them per destination row to balance the two groups (minimizes the
    # padded ELL widths, which the Q7 descriptor-gen time scales with).
    lo_forced = tabrow < HIBASE
    hi_forced = tabrow >= LOWMAX
    flex = ~(lo_forced | hi_forced)

    nlo = np.bincount(new_row[lo_forced], minlength=NTOT)
    nhi = np.bincount(new_row[hi_forced], minlength=NTOT)
    nflex = np.bincount(new_row[flex], minlength=NTOT)
    x = np.clip((nhi + nflex - nlo + 1) // 2, 0, nflex)  # flex sent to LOW

    # rank of each flex edge within its row's flex group
    fsel = np.nonzero(flex)[0]
    fo = np.argsort(new_row[fsel], kind="stable")
    fstarts = np.zeros(NTOT + 1, np.int64)
    fstarts[1:] = np.cumsum(nflex)
    frank = np.arange(len(fsel)) - fstarts[new_row[fsel][fo]]
    flex_low = np.zeros(N_EDGES, bool)
    flex_low[fsel[fo]] = frank < x[new_row[fsel][fo]]

    is_low = lo_forced | flex_low

    def build_half(mask, values, pad):
        cnt = np.bincount(new_row[mask], minlength=NTOT)
        W = cnt.reshape(NC, TPC, P).max(axis=(0, 2)).astype(np.int64)
        W = np.maximum(W, 1)
        off = np.zeros(TPC + 1, np.int64)
        off[1:] = np.cumsum(W)
        starts = np.zeros(NTOT + 1, np.int64)
        starts[1:] = np.cumsum(cnt)
        sel = np.nonzero(mask)[0]
        o = np.argsort(new_row[sel], kind="stable")
        nr = new_row[sel][o]
        vv = values[sel][o]
        rank = np.arange(len(sel)) - starts[nr]
        core_e = nr // RPC
        rem = nr % RPC
        ti = rem // P
        p_e = rem % P
        colpad = np.full((NC, P, int(off[-1])), pad, np.int32)
        colpad[core_e, p_e, off[ti] + rank] = vv
        return W, off, colpad

    # --- chunked edge lists (segmented PE reduce; no per-row ELL padding) ---
    # Per (core, tile, side): the edge list sorted by row, padded to a
    # multiple of 128 (chunk). Chunk counts are shared across cores (SPMD):
    # ch_lo[t] = max over cores of ceil(cnt_lo/128).
    def side_lists(mask, pad_idx):
        cnt = np.bincount(new_row[mask], minlength=NTOT).reshape(NC, TPC, P)
        cnt_t = cnt.sum(axis=2)  # [NC, TPC] edges per (core, tile)
        ch = -(-cnt_t.max(axis=0) // P)  # [TPC] chunks per tile (shared)
        sel = np.nonzero(mask)[0]
        o = np.argsort(new_row[sel], kind="stable")
        return cnt_t, ch, sel[o]

    cnt_lo, ch_lo, elist_lo = side_lists(is_low, 0)
    cnt_hi, ch_hi, elist_hi = side_lists(~is_low, HIPAD)

    dinv_new = np.zeros(NTOT, np.float32)
    dinv_new[old2new] = dinv
    dinv_t = np.ascontiguousarray(dinv_new.reshape(NC, TPC, P).transpose(0, 2, 1))

    # batches of consecutive tiles with sum(ch_lo+ch_hi) <= WBMAX chunks
    ch_tot = ch_lo + ch_hi
    batches = []
    i = 0
    while i < TPC:
        j = i + 1
        while j < TPC and ch_tot[i : j + 1].sum() <= WBMAX:
            j += 1
        batches.append((i, j))
        i = j

    # per-(core,tile,side) padded flat lists of table rows + row-in-tile
    def build_padded(elist, cnt_t, ch, values, pad_idx):
        # elist: edge ids sorted by new_row; values: per-edge table index
        nr = new_row[elist]
        core_e = nr // RPC
        rem = nr % RPC
        ti = rem // P
        p_e = rem % P
        # position within (core, tile) group
        starts = np.zeros(NC * TPC + 1, np.int64)
        starts[1:] = np.cumsum(cnt_t.reshape(-1))
        gid = core_e * TPC + ti
        pos = np.arange(len(elist)) - starts[gid]
        # output offsets: chunks are per-tile, padded length ch[t]*128
        choff = np.zeros(TPC + 1, np.int64)
        choff[1:] = np.cumsum(ch)
        L = int(choff[-1]) * P  # padded edges per core
        flat = np.full((NC, L), pad_idx, np.int64)
        rr = np.zeros((NC, L), np.int64)
        dst = choff[ti] * P + pos
        flat[core_e, dst] = values[elist]
        rr[core_e, dst] = p_e
        return flat, rr, choff

    flat_lo, rr_lo, choff_lo = build_padded(elist_lo, cnt_lo, ch_lo, tabrow, 0)
    flat_hi, rr_hi, choff_hi = build_padded(
        elist_hi, cnt_hi, ch_hi, tabrow - HIBASE, HIPAD
    )

    # assemble per-batch idx images + global rowrel array (G-local chunk order)
    tot_s = int((choff_lo[-1] + choff_hi[-1])) * 8
    nch_tot = int(choff_lo[-1] + choff_hi[-1])
    idx_img = np.zeros((NC, P, tot_s), np.int16)
    rowrel = np.zeros((NC, P, nch_tot), np.float32)
    binfo = []  # (t0, t1, CL, CH, soff, gch0)
    soff = 0
    gch = 0
    for t0, t1 in batches:
        CL = int(choff_lo[t1] - choff_lo[t0])
        CH = int(choff_hi[t1] - choff_hi[t0])
        for c in range(NC):
            fl = flat_lo[c][choff_lo[t0] * P : choff_lo[t1] * P]
            fh = flat_hi[c][choff_hi[t0] * P : choff_hi[t1] * P]
            if CL:
                idx_img[c][:, soff : soff + CL * 8] = _wrap_idx(fl)
                rowrel[c][:, gch : gch + CL] = (
                    rr_lo[c][choff_lo[t0] * P : choff_lo[t1] * P]
                    .reshape(CL, P)
                    .T
                )
            if CH:
                idx_img[c][:, soff + CL * 8 : soff + (CL + CH) * 8] = _wrap_idx(fh)
                rowrel[c][:, gch + CL : gch + CL + CH] = (
                    rr_hi[c][choff_hi[t0] * P : choff_hi[t1] * P]
                    .reshape(CH, P)
                    .T
                )
        binfo.append((t0, t1, CL, CH, soff, gch))
        soff += (CL + CH) * 8
        gch += CL + CH
    assert soff == tot_s and gch == nch_tot

    return dict(
        dinv=dinv,
        old2new=old2new,
        dinv_t=dinv_t,
        ch_lo=ch_lo,
        ch_hi=ch_hi,
        choff_lo=choff_lo,
        choff_hi=choff_hi,
        idx_img=idx_img,
        rowrel=rowrel,
        binfo=binfo,
        tot_s=tot_s,
        nch_tot=nch_tot,
    )


def _build_graph(pre):
    from concourse import bacc, mybir, tile
    import concourse.bass as bass
    from concourse.masks import make_identity

    f32 = mybir.dt.float32
    i16 = mybir.dt.int16
    ch_lo, ch_hi = pre["ch_lo"], pre["ch_hi"]
    choff_lo, choff_hi = pre["choff_lo"], pre["choff_hi"]
    binfo = pre["binfo"]
    tot_s = pre["tot_s"]
    nch_tot = pre["nch_tot"]

    nc = bacc.Bacc(None, target_bir_lowering=False, num_devices=NC)

    x_in = nc.declare_dram_parameter("x", [RPC, F], f32, isOutput=False)
    idx_in = nc.declare_dram_parameter("idximg", [P, tot_s], i16, isOutput=False)
    rowrel_in = nc.declare_dram_parameter("rowrel", [P, nch_tot], f32, isOutput=False)
    iota_in = nc.declare_dram_parameter("iotarep", [P, P], f32, isOutput=False)
    dinvt_in = nc.declare_dram_parameter("dinvt", [P, TPC], f32, isOutput=False)
    chebw_in = nc.declare_dram_parameter("cheb_w", [K, F, H], f32, isOutput=False)
    chebb_in = nc.declare_dram_parameter("cheb_b", [H, 1], f32, isOutput=False)
    finw_in = nc.declare_dram_parameter("final_w", [H, 1], f32, isOutput=False)
    finb_in = nc.declare_dram_parameter("final_b", [1, 1], f32, isOutput=False)
    y_out = nc.declare_dram_parameter("y", [1, RPC], f32, isOutput=True)

    tabA = nc.dram_tensor("tabA", [NTAB, F], f32, addr_space="Shared")
    tabB = nc.dram_tensor("tabB", [NTAB, F], f32, addr_space="Shared")
    tab_in = nc.dram_tensor("tabin", [RPC, F], f32)

    if DEBUG_DUMPS:
        dbg_tab1 = nc.declare_dram_parameter("dbg_tab1", [NTAB, F], f32, isOutput=True)
        dbg_g = nc.declare_dram_parameter("dbg_g", [P, WBMAX * F], f32, isOutput=True)
        dbg_tx1 = nc.declare_dram_parameter("dbg_tx1", [P, TPC * F], f32, isOutput=True)

    rg = [list(range(NC))]

    with tile.TileContext(nc) as tc:
        with (
            tc.tile_pool(name="persist", bufs=1) as persist,
            tc.tile_pool(name="work", bufs=4) as work,
            tc.tile_pool(name="gpool", bufs=3) as gpool,
            tc.tile_pool(name="psum", bufs=2, space="PSUM") as psum,
            tc.tile_pool(name="psum_s", bufs=3, space="PSUM") as psum_s,
            tc.tile_pool(name="psum_h", bufs=2, space="PSUM") as psum_h,
            tc.tile_pool(name="psum_y", bufs=1, space="PSUM") as psum_y,
        ):
            # ---- persistent tiles ----
            ident = persist.tile([P, P], f32)
            make_identity(nc, ident[:])

            dinv_sb = persist.tile([P, TPC], f32)
            nc.sync.dma_start(out=dinv_sb[:], in_=dinvt_in[:, :])
            rowrel_sb = persist.tile([P, nch_tot], f32)
            nc.sync.dma_start(out=rowrel_sb[:], in_=rowrel_in[:, :])
            iota_sb = persist.tile([P, P], f32)
            nc.sync.dma_start(out=iota_sb[:], in_=iota_in[:, :])
            ndinv_sb = persist.tile([P, TPC], f32)
            nc.vector.tensor_scalar_mul(ndinv_sb[:], dinv_sb[:], -1.0)
            n2dinv_sb = persist.tile([P, TPC], f32)
            nc.vector.tensor_scalar_mul(n2dinv_sb[:], dinv_sb[:], -2.0)

            wc01 = persist.tile([2 * F, H], f32)
            nc.sync.dma_start(out=wc01[0:F, :], in_=chebw_in[0])
            nc.sync.dma_start(out=wc01[F : 2 * F, :], in_=chebw_in[1])
            wc23 = persist.tile([2 * F, H], f32)
            nc.sync.dma_start(out=wc23[0:F, :], in_=chebw_in[2])
            nc.sync.dma_start(out=wc23[F : 2 * F, :], in_=chebw_in[3])

            chebb_sb = persist.tile([H, 1], f32)
            nc.sync.dma_start(out=chebb_sb[:], in_=chebb_in[:, :])
            finw_sb = persist.tile([H, 1], f32)
            nc.sync.dma_start(out=finw_sb[:], in_=finw_in[:, :])
            finb_sb = persist.tile([1, 1], f32)
            nc.sync.dma_start(out=finb_sb[:], in_=finb_in[:, :])

            # feature-major stashes of Tx_k^T for the final projection
            txt01 = persist.tile([P, TPC * P], f32)  # parts 0:64 Tx0^T, 64:128 Tx1^T
            txt23 = persist.tile([P, TPC * P], f32)
            # node-major Tx1 (needed by the k=3 recursion)
            tx1slab = persist.tile([P, TPC * F], f32)
            # final output row
            y_sb = persist.tile([1, RPC], f32)

            # ---- zero rows of both tables (row 0 and the tail) ----
            zt = work.tile([P, F], f32, tag="xt")
            nc.vector.memset(zt[:], 0.0)
            for tab in (tabA, tabB):
                nc.sync.dma_start(out=tab[0:1, :], in_=zt[0:1, :])
                nc.sync.dma_start(out=tab[NTOT + 1 : NTAB, :], in_=zt[0:P, :])

            # ---- step 0: table1 = dinv*x, stash Tx0^T ----
            for i in range(TPC):
                rows = slice(i * P, (i + 1) * P)
                xt = work.tile([P, F], f32, tag="xt")
                nc.sync.dma_start(out=xt[:], in_=x_in[rows, :])
                tabt = work.tile([P, F], f32, tag="tabt")
                nc.scalar.activation(
                    out=tabt[:],
                    in_=xt[:],
                    func=mybir.ActivationFunctionType.Copy,
                    scale=dinv_sb[:, i : i + 1],
                )
                nc.sync.dma_start(out=tab_in[rows, :], in_=tabt[:])
                ps = psum.tile([F, P], f32)
                nc.tensor.transpose(out=ps[:], in_=xt[:], identity=ident[:])
                nc.scalar.activation(
                    out=txt01[0:F, rows],
                    in_=ps[:],
                    func=mybir.ActivationFunctionType.Copy,
                )

            def allgather(dst):
                nc.gpsimd.collective_compute(
                    "AllGather",
                    mybir.AluOpType.bypass,
                    replica_groups=rg,
                    ins=[tab_in[:, :].opt()],
                    outs=[dst[1 : NTOT + 1, :].opt()],
                )

            allgather(tabA)

            if DEBUG_DUMPS:
                nc.sync.dma_start(out=dbg_tab1[:, :], in_=tabA[:, :])

            # ---- chebyshev steps ----
            for s in (1, 2, 3):
                tab = tabA if s != 2 else tabB
                for t0, t1, CL, CH, soff, gch0 in binfo:
                    sb_cols = (CL + CH) * 8
                    idxt = work.tile([P, WBMAX * 8], i16, tag="idx")
                    nc.sync.dma_start(
                        out=idxt[:, :sb_cols], in_=idx_in[:, soff : soff + sb_cols]
                    )
                    G = gpool.tile([P, WBMAX * F], f32, tag="G")
                    if CL:
                        nc.gpsimd.dma_gather(
                            out_ap=G[:, : CL * F].rearrange("p (c f) -> p c f", f=F),
                            in_ap=tab[:, :],
                            idxs_ap=idxt[:, : CL * 8],
                            num_idxs=CL * P,
                            num_idxs_reg=CL * P,
                            elem_size=F,
                            single_packet=False,
                        )
                    if CH:
                        nc.gpsimd.dma_gather(
                            out_ap=G[:, CL * F : (CL + CH) * F].rearrange(
                                "p (c f) -> p c f", f=F
                            ),
                            in_ap=tab[HIBASE:NTAB, :],
                            idxs_ap=idxt[:, CL * 8 : (CL + CH) * 8],
                            num_idxs=CH * P,
                            num_idxs_reg=CH * P,
                            elem_size=F,
                            single_packet=False,
                        )
                    if DEBUG_DUMPS and s == 1 and t0 == 0:
                        nc.sync.dma_start(
                            out=dbg_g[:, : (CL + CH) * F], in_=G[:, : (CL + CH) * F]
                        )
                    for i in range(t0, t1):
                        # this tile's chunk columns within G (low then high)
                        lo0 = int(choff_lo[i] - choff_lo[t0])
                        nlo = int(ch_lo[i])
                        hi0 = CL + int(choff_hi[i] - choff_hi[t0])
                        nhi = int(ch_hi[i])
                        gl = [lo0 + j for j in range(nlo)] + [
                            hi0 + j for j in range(nhi)
                        ]
                        rows = slice(i * P, (i + 1) * P)
                        fcols = slice(i * F, (i + 1) * F)
                        S_ps = psum_s.tile([P, F], f32)
                        for j, c in enumerate(gl):
                            oneh = work.tile([P, P], f32, tag="oneh")
                            nc.vector.tensor_scalar(
                                out=oneh[:],
                                in0=iota_sb[:],
                                scalar1=rowrel_sb[:, gch0 + c : gch0 + c + 1],
                                scalar2=None,
                                op0=mybir.AluOpType.is_equal,
                            )
                            nc.tensor.matmul(
                                out=S_ps[:],
                                lhsT=oneh[:],
                                rhs=G[:, c * F : (c + 1) * F],
                                start=(j == 0),
                                stop=(j == len(gl) - 1),
                            )
                        if s == 1:
                            nc.vector.tensor_scalar_mul(
                                tx1slab[:, fcols], S_ps[:], ndinv_sb[:, i : i + 1]
                            )
                            txk = tx1slab[:, fcols]
                        else:
                            tmp = work.tile([P, F], f32, tag="tmp")
                            nc.vector.tensor_scalar_mul(
                                tmp[:], S_ps[:], n2dinv_sb[:, i : i + 1]
                            )
                            txk_t = work.tile([P, F], f32, tag="txk")
                            if s == 2:
                                xt = work.tile([P, F], f32, tag="xt")
                                nc.sync.dma_start(out=xt[:], in_=x_in[rows, :])
                                nc.vector.tensor_sub(txk_t[:], tmp[:], xt[:])
                            else:
                                nc.vector.tensor_sub(
                                    txk_t[:], tmp[:], tx1slab[:, fcols]
                                )
                            txk = txk_t[:]
                        if s < 3:
                            tabt = work.tile([P, F], f32, tag="tabt")
                            nc.scalar.activation(
                                out=tabt[:],
                                in_=txk,
                                func=mybir.ActivationFunctionType.Copy,
                                scale=dinv_sb[:, i : i + 1],
                            )
                            nc.sync.dma_start(out=tab_in[rows, :], in_=tabt[:])
                        ps = psum.tile([F, P], f32)
                        nc.tensor.transpose(out=ps[:], in_=txk, identity=ident[:])
                        dst = txt01 if s == 1 else txt23
                        pr = slice(F, 2 * F) if s in (1, 3) else slice(0, F)
                        nc.scalar.activation(
                            out=dst[pr, rows],
                            in_=ps[:],
                            func=mybir.ActivationFunctionType.Copy,
                        )

                if s == 1:
                    allgather(tabB)
                elif s == 2:
                    allgather(tabA)

            if DEBUG_DUMPS:
                nc.sync.dma_start(out=dbg_tx1[:, :], in_=tx1slab[:])

            # ---- projection + tanh + final linear ----
            for i in range(TPC):
                rows = slice(i * P, (i + 1) * P)
                hps = psum_h.tile([H, P], f32)
                nc.tensor.matmul(
                    out=hps[:], lhsT=wc01[:], rhs=txt01[:, rows], start=True, stop=False
                )
                nc.tensor.matmul(
                    out=hps[:], lhsT=wc23[:], rhs=txt23[:, rows], start=False, stop=True
                )
                hT = work.tile([H, P], f32, tag="hT")
                nc.scalar.activation(
                    out=hT[:],
                    in_=hps[:],
                    func=mybir.ActivationFunctionType.Tanh,
                    bias=chebb_sb[:, 0:1],
                    scale=1.0,
                )
                yps = psum_y.tile([1, P], f32, tag="yps")  # 1 bank is enough
                nc.tensor.matmul(
                    out=yps[:], lhsT=finw_sb[:], rhs=hT[:], start=True, stop=True
                )
                nc.vector.tensor_scalar_add(y_sb[0:1, rows], yps[:], finb_sb[0:1, 0:1])

            nc.sync.dma_start(out=y_out[:, :], in_=y_sb[:])

    nc.finalize()
    return nc


def run(features, edge_index, cheb_w, cheb_b, final_w, final_b, **spmd_kwargs):
    """Build + compile + run; returns (y, BassKernelResults)."""
    from concourse.bass_utils import run_bass_kernel_spmd

    features = np.asarray(features, np.float32)
    edge_index = np.asarray(edge_index)
    cheb_w = np.asarray(cheb_w, np.float32)
    cheb_b = np.asarray(cheb_b, np.float32)
    final_w = np.asarray(final_w, np.float32)
    final_b = np.asarray(final_b, np.float32)

    pre = _preprocess(edge_index)
    nc = _build_graph(pre)

    old2new = pre["old2new"]
    x_new = np.zeros((NTOT, F), np.float32)
    x_new[old2new] = features
    x_new = x_new.reshape(NC, RPC, F)

    iota = np.tile(np.arange(P, dtype=np.float32), (P, 1))
    in_maps = []
    for c in range(NC):
        in_maps.append(
            dict(
                x=np.ascontiguousarray(x_new[c]),
                idximg=np.ascontiguousarray(pre["idx_img"][c]),
                rowrel=np.ascontiguousarray(pre["rowrel"][c]),
                iotarep=iota,
                dinvt=np.ascontiguousarray(pre["dinv_t"][c]),
                cheb_w=cheb_w,
                cheb_b=cheb_b.reshape(H, 1),
                final_w=final_w.reshape(H, 1),
                final_b=final_b.reshape(1, 1),
            )
        )

    res = run_bass_kernel_spmd(nc, in_maps, core_ids=list(range(NC)), **spmd_kwargs)
    y_new = np.concatenate([r["y"].reshape(-1) for r in res.results])
    return y_new[old2new].astype(np.float32), res


def kernel(features, edge_index, cheb_w, cheb_b, final_w, final_b):
    y, _ = run(features, edge_index, cheb_w, cheb_b, final_w, final_b)
    return y
